# revision 1
# baseline (speedup 1.0000x reference)
"""Trainium2 kernel for DisplacementVectorsASU — group-block gather design.

Each dma_gather descriptor costs ~23ns of DMA-engine time and reads a
256B minimum element, so per-edge gathers (baseline) cost ~35us/tile.
Instead: host sorts edges by the node being gathered, packs G=32
consecutive sorted edges into one "group" that shares a single 4-node
256B table block (table rows padded to 64B: [x,y,z,1,0*12]); one
descriptor serves 32 edges. A per-edge 1-of-4 select (tensor_copy +
3 one-hot copy_predicated with int8 mask planes) recovers the exact
f32 node row on-chip. All elementwise math runs on DVE (Pool/GpSimd
tensor ops are Q7 software, ~5.6us/op); symmops stream j-major so
every A(j) slice is contiguous; only rows 0..2 of each symmop are
streamed (row 3 is unused by the reference einsum).

Two passes (both node orders can't be sorted at once):
  A (dst-sorted): gather out4, w = wrap(symm[:, :3, :] @ out4) + trans,
    write w as bf16 (|err| <= ~0.03 vs 0.2 abs tolerance).
  B (src-sorted): gather in3, res = in3 - w (w re-permuted by host
    between launches, like all other host-side data marshalling here).

Group g of tile t lands at partition g%128, slots (g//128)*G..+G-1, and
its gathered block lands at partition g%128, group-slot g//128 — same
partition, so the select needs no cross-partition traffic.
"""
import sys

sys.path.insert(0, "/opt/trn_rl_repo")

import numpy as np

try:
    from ml_dtypes import bfloat16 as np_bf16
except Exception:  # pragma: no cover
    np_bf16 = None

N_NODES = 100_000
M_TOTAL = 4_000_000
N_CORES = 8
M_CORE = M_TOTAL // N_CORES

P = 128
K = 256                 # slots per partition per tile
G = 32                  # edges per group (per gather descriptor)
H = K // G              # group-slots per partition
GPT = P * H             # groups (gather descriptors) per tile = 1024
TILE = P * K            # 32768 edges per tile
NBLK = N_NODES // 4     # 4-node 256B blocks in the table

USE_MOD = False         # python_mod fails DVE codegen; use round-to-nearest
RND = 12582912.0        # 1.5 * 2^23 fp32 rounding constant
W_BF16 = True           # write/read the intermediate w as bf16
if np_bf16 is None:     # keep host/device dtypes in lockstep
    W_BF16 = False

_cache = {}

LAST_NT_A = None
LAST_NT_B = None


def _mybir():
    import concourse.mybir as mybir
    return mybir


def emit_wrap(nc, eng, out_ap, acc_ap, tr_ap, pool):
    """out = (acc - floor(acc)) + tr, exactly."""
    mybir = _mybir()
    op = mybir.AluOpType
    if USE_MOD:
        f32 = mybir.dt.float32
        w = pool.tile(acc_ap.shape, f32, name="wtmp")
        eng.tensor_scalar(out=w[:], in0=acc_ap, scalar1=1.0, scalar2=None,
                          op0=op.python_mod)
        eng.tensor_tensor(out=out_ap, in0=w[:], in1=tr_ap, op=op.add)
    else:
        f32 = mybir.dt.float32
        y = pool.tile(acc_ap.shape, f32, name="y")
        w = pool.tile(acc_ap.shape, f32, name="w")
        msk = pool.tile(acc_ap.shape, f32, name="msk")
        eng.tensor_scalar(out=y[:], in0=acc_ap, scalar1=RND, scalar2=-RND,
                          op0=op.add, op1=op.add)
        eng.tensor_tensor(out=w[:], in0=acc_ap, in1=y[:], op=op.subtract)
        eng.tensor_scalar(out=msk[:], in0=w[:], scalar1=0.0, scalar2=None,
                          op0=op.is_lt)
        eng.tensor_tensor(out=w[:], in0=w[:], in1=msk[:], op=op.add)
        eng.tensor_tensor(out=out_ap, in0=w[:], in1=tr_ap, op=op.add)


def emit_select(nc, pool, c, mk, width, out):
    """1-of-4 select: out[(h g), 0:width] = c[h, 16*r:16*r+width] where
    mk[j] = (r == j+1); c is (P, H, 64), mk is (P, 3, K, 1).

    4 DVE ops: copy candidate 0, then one-hot predicated overwrites."""
    v = nc.vector

    def cand(r):
        a = c[:, :, 16 * r:16 * r + width]
        return a.unsqueeze(2).broadcast_to((P, H, G, width))

    def m(j):
        return mk[:, j].broadcast_to((P, K, width)).rearrange(
            "p (h g) c -> p h g c", h=H)

    out4 = out[:].rearrange("p (h g) c -> p h g c", h=H)
    v.tensor_copy(out4, cand(0))
    v.copy_predicated(out4, m(0), cand(1))
    v.copy_predicated(out4, m(1), cand(2))
    v.copy_predicated(out4, m(2), cand(3))


def emit_tile_a(nc, pool, t, tbl_d, idx_d, mk_d, symm_d, tr_d, w_d):
    """Pass A tile: gather out4 (dst-sorted), w = wrap(symm@out4)+tr."""
    mybir = _mybir()
    f32, i16, bf16 = mybir.dt.float32, mybir.dt.int16, mybir.dt.bfloat16
    op = mybir.AluOpType

    idx = pool.tile((P, GPT // 16), i16, name="idx")
    mk = pool.tile((P, 3, K, 1), mybir.dt.int8, name="mk")
    c = pool.tile((P, H, 64), f32, name="c")
    s = pool.tile((P, K, 12), f32, name="s")
    tr = pool.tile((P, K, 3), f32, name="tr")
    sel = pool.tile((P, K, 3), f32, name="sel")
    acc = pool.tile((P, K, 3), f32, name="acc")
    tmp1 = pool.tile((P, K, 3), f32, name="tmp1")
    tmp2 = pool.tile((P, K, 3), f32, name="tmp2")
    wout = pool.tile((P, K, 3), bf16 if W_BF16 else f32, name="wout")

    nc.sync.dma_start(idx[:], idx_d[t])
    nc.sync.dma_start(mk[:].rearrange("p a k c -> p (a k c)"), mk_d[t])
    nc.sync.dma_start(s[:].rearrange("p k c -> p (k c)"), symm_d[t])
    nc.sync.dma_start(tr[:].rearrange("p k c -> p (k c)"), tr_d[t])
    nc.gpsimd.dma_gather(
        out_ap=c[:], in_ap=tbl_d[:], idxs_ap=idx[:],
        num_idxs=GPT, num_idxs_reg=GPT, elem_size=64, single_packet=False)

    # the gathered 4th component ("1") is never used: select only x,y,z
    emit_select(nc, pool, c, mk, 3, sel)

    # acc[c] = ((A0*G0 + A3) + A1*G1) + A2*G2. All compute on DVE: the
    # Pool/GpSimd engine runs tensor ops in Q7 software (~5.6us per op
    # measured) so it only does gather descriptor-gen here. symm is
    # j-major (P, K, j=4, c=3) so every A(j) slice is contiguous —
    # strided last-dim reads measured ~2x slower end-to-end.
    v = nc.vector
    s4 = s[:].rearrange("p k (j c) -> p k j c", j=4)
    A = lambda j: s4[:, :, j, :]
    Gj = lambda j: sel[:, :, j:j + 1].to_broadcast((P, K, 3))
    v.tensor_tensor(out=tmp1[:], in0=A(1), in1=Gj(1), op=op.mult)
    v.tensor_tensor(out=tmp2[:], in0=A(2), in1=Gj(2), op=op.mult)
    v.tensor_tensor(out=acc[:], in0=A(0), in1=Gj(0), op=op.mult)
    v.tensor_tensor(out=acc[:], in0=acc[:], in1=A(3), op=op.add)
    v.tensor_tensor(out=acc[:], in0=acc[:], in1=tmp1[:], op=op.add)
    v.tensor_tensor(out=acc[:], in0=acc[:], in1=tmp2[:], op=op.add)
    v.tensor_scalar(out=tmp1[:], in0=acc[:], scalar1=RND, scalar2=-RND,
                    op0=op.add, op1=op.add)
    v.tensor_tensor(out=acc[:], in0=acc[:], in1=tmp1[:], op=op.subtract)
    v.tensor_scalar(out=tmp2[:], in0=acc[:], scalar1=0.0, scalar2=None,
                    op0=op.is_lt)
    v.tensor_tensor(out=acc[:], in0=acc[:], in1=tmp2[:], op=op.add)
    v.tensor_tensor(out=wout[:], in0=acc[:], in1=tr[:], op=op.add)
    nc.sync.dma_start(w_d[t], wout[:].rearrange("p k c -> p (k c)"))


def emit_tile_b(nc, pool, t, tbl_d, idx_d, mk_d, win_d, out_d):
    """Pass B tile: gather in3 (src-sorted), res = in3 - w."""
    mybir = _mybir()
    f32, i16, bf16 = mybir.dt.float32, mybir.dt.int16, mybir.dt.bfloat16
    op = mybir.AluOpType

    idx = pool.tile((P, GPT // 16), i16, name="idx")
    mk = pool.tile((P, 3, K, 1), mybir.dt.int8, name="mk")
    c = pool.tile((P, H, 64), f32, name="c")
    win = pool.tile((P, K, 3), bf16 if W_BF16 else f32, name="win")
    sel = pool.tile((P, K, 3), f32, name="sel3")
    res = pool.tile((P, K, 3), f32, name="res")

    nc.sync.dma_start(idx[:], idx_d[t])
    nc.sync.dma_start(mk[:].rearrange("p a k c -> p (a k c)"), mk_d[t])
    nc.sync.dma_start(win[:].rearrange("p k c -> p (k c)"), win_d[t])
    nc.gpsimd.dma_gather(
        out_ap=c[:], in_ap=tbl_d[:], idxs_ap=idx[:],
        num_idxs=GPT, num_idxs_reg=GPT, elem_size=64, single_packet=False)

    emit_select(nc, pool, c, mk, 3, sel)
    nc.vector.tensor_tensor(out=res[:], in0=sel[:], in1=win[:],
                            op=op.subtract)
    nc.sync.dma_start(out_d[t], res[:].rearrange("p k c -> p (k c)"))


def _build_a(nt):
    key = ("a", nt)
    if key in _cache:
        return _cache[key]
    mybir = _mybir()
    import concourse.tile as tile
    from concourse import bacc
    f32, i16, bf16 = mybir.dt.float32, mybir.dt.int16, mybir.dt.bfloat16

    nc = bacc.Bacc(None, target_bir_lowering=False, debug=False)
    tbl_d = nc.dram_tensor("tbl", (NBLK, 64), f32, kind="ExternalInput")
    idx_d = nc.dram_tensor("idx", (nt, P, GPT // 16), i16, kind="ExternalInput")
    mk_d = nc.dram_tensor("mk", (nt, P, 3 * K), mybir.dt.int8, kind="ExternalInput")
    symm_d = nc.dram_tensor("symm", (nt, P, K * 12), f32, kind="ExternalInput")
    tr_d = nc.dram_tensor("tr", (nt, P, K * 3), f32, kind="ExternalInput")
    w_d = nc.dram_tensor("w", (nt, P, K * 3), bf16 if W_BF16 else f32,
                         kind="ExternalOutput")
    with tile.TileContext(nc) as tc:
        with tc.tile_pool(name="pool", bufs=4) as pool:
            for t in range(nt):
                emit_tile_a(nc, pool, t, tbl_d, idx_d, mk_d, symm_d, tr_d, w_d)
    nc.compile()
    _cache[key] = nc
    return nc


def _build_b(nt):
    key = ("b", nt)
    if key in _cache:
        return _cache[key]
    mybir = _mybir()
    import concourse.tile as tile
    from concourse import bacc
    f32, i16, bf16 = mybir.dt.float32, mybir.dt.int16, mybir.dt.bfloat16

    nc = bacc.Bacc(None, target_bir_lowering=False, debug=False)
    tbl_d = nc.dram_tensor("tbl", (NBLK, 64), f32, kind="ExternalInput")
    idx_d = nc.dram_tensor("idx", (nt, P, GPT // 16), i16, kind="ExternalInput")
    mk_d = nc.dram_tensor("mk", (nt, P, 3 * K), mybir.dt.int8, kind="ExternalInput")
    win_d = nc.dram_tensor("win", (nt, P, K * 3), bf16 if W_BF16 else f32,
                           kind="ExternalInput")
    out_d = nc.dram_tensor("out", (nt, P, K * 3), f32, kind="ExternalOutput")
    with tile.TileContext(nc) as tc:
        with tc.tile_pool(name="pool", bufs=4) as pool:
            for t in range(nt):
                emit_tile_b(nc, pool, t, tbl_d, idx_d, mk_d, win_d, out_d)
    nc.compile()
    _cache[key] = nc
    return nc


def _make_table(frac_coords):
    tbl = np.zeros((N_NODES, 16), np.float32)
    tbl[:, :3] = frac_coords
    tbl[:, 3] = 1.0
    return np.ascontiguousarray(tbl.reshape(NBLK, 64))


def _pack_core(ids):
    """Group-pack one core's sorted node ids.

    Returns (gid, pos, blk_per_group, n_groups): per-edge group id and
    within-group slot; per-group table block index.
    """
    m = ids.shape[0]
    blk = ids >> 2
    newrun = np.empty(m, np.bool_)
    newrun[0] = True
    np.not_equal(blk[1:], blk[:-1], out=newrun[1:])
    run_id = np.cumsum(newrun) - 1
    run_start = np.flatnonzero(newrun)
    t_in = np.arange(m, dtype=np.int64) - run_start[run_id]
    lengths = np.diff(np.append(run_start, m))
    gpr = (lengths + G - 1) // G
    goff = np.concatenate([[0], np.cumsum(gpr)])
    gid = goff[run_id] + t_in // G
    pos = t_in % G
    ng = int(goff[-1])
    blk_per_group = np.zeros(ng, np.int16)
    blk_per_group[gid] = blk.astype(np.int16)
    return gid, pos, blk_per_group, ng


def _layout_core(ids, nt):
    """Flat device positions (into (nt*P, K) slots) + per-tile idx table +
    mask planes for one core's sorted node ids."""
    gid, pos, blkg, ng = _pack_core(ids)
    ngpad = nt * GPT
    assert ng <= ngpad, (ng, ngpad)
    tile_ = gid // GPT
    gl = gid % GPT
    part = gl % P
    h = gl // P
    fpos = (tile_ * P + part) * K + h * G + pos

    idxg = np.zeros(ngpad, np.int16)
    idxg[:ng] = blkg
    iw = np.tile(
        idxg.reshape(nt, GPT // 16, 16).transpose(0, 2, 1), (1, 8, 1))

    r = (ids & 3).astype(np.int8)
    mk_flat = np.zeros((3, nt * P * K), np.int8)
    for j in range(3):
        mk_flat[j, fpos] = (r == j + 1)
    mk = mk_flat.reshape(3, nt * P, K).transpose(1, 0, 2)
    return fpos, np.ascontiguousarray(iw), \
        np.ascontiguousarray(mk.reshape(nt, P, 3 * K))


def _nt_for(ids_by_core):
    return max(
        ( _pack_core(ids)[3] + GPT - 1) // GPT for ids in ids_by_core)


def kernel(frac_coords, edge_indices, symmops, cell_translations):
    global LAST_NT_A, LAST_NT_B
    from concourse.bass_utils import run_bass_kernel_spmd

    frac = np.asarray(frac_coords, np.float32)
    symm = np.asarray(symmops, np.float32)
    trans = np.asarray(cell_translations, np.float32)
    src = np.asarray(edge_indices[0], np.int64)
    dst = np.asarray(edge_indices[1], np.int64)
    tbl = _make_table(frac)
    w_dt = np_bf16 if (W_BF16 and np_bf16 is not None) else np.float32

    # ---------------- pass A: dst-sorted ----------------
    orderA = np.argsort(dst, kind="stable")
    dA = dst[orderA]
    idsA = [dA[c * M_CORE:(c + 1) * M_CORE] for c in range(N_CORES)]
    nta = _nt_for(idsA)
    LAST_NT_A = nta
    ncA = _build_a(nta)

    in_maps = []
    fposA = []
    # j-major: (M, j=4, c=3) so device A(j) slices are contiguous
    symmA = np.ascontiguousarray(
        symm[orderA][:, :3, :].transpose(0, 2, 1)).reshape(M_TOTAL, 12)
    trA = trans[orderA]
    for c in range(N_CORES):
        fpos, iw, mk = _layout_core(idsA[c], nta)
        fposA.append(fpos)
        sl = slice(c * M_CORE, (c + 1) * M_CORE)
        sdev = np.zeros((nta * P * K, 12), np.float32)
        sdev[fpos] = symmA[sl]
        tdev = np.zeros((nta * P * K, 3), np.float32)
        tdev[fpos] = trA[sl]
        in_maps.append({
            "tbl": tbl,
            "idx": iw,
            "mk": mk,
            "symm": sdev.reshape(nta, P, K * 12),
            "tr": tdev.reshape(nta, P, K * 3),
        })
    resA = run_bass_kernel_spmd(ncA, in_maps, list(range(N_CORES)))

    w_orig = np.empty((M_TOTAL, 3), w_dt)
    for c in range(N_CORES):
        wdev = np.asarray(resA.results[c]["w"]).reshape(nta * P * K, 3)
        w_orig[orderA[c * M_CORE:(c + 1) * M_CORE]] = wdev[fposA[c]]

    # ---------------- pass B: src-sorted ----------------
    orderB = np.argsort(src, kind="stable")
    sB = src[orderB]
    idsB = [sB[c * M_CORE:(c + 1) * M_CORE] for c in range(N_CORES)]
    ntb = _nt_for(idsB)
    LAST_NT_B = ntb
    ncB = _build_b(ntb)

    in_maps = []
    fposB = []
    wB = w_orig[orderB]
    for c in range(N_CORES):
        fpos, iw, mk = _layout_core(idsB[c], ntb)
        fposB.append(fpos)
        sl = slice(c * M_CORE, (c + 1) * M_CORE)
        wdev = np.zeros((ntb * P * K, 3), w_dt)
        wdev[fpos] = wB[sl]
        in_maps.append({
            "tbl": tbl,
            "idx": iw,
            "mk": mk,
            "win": wdev.reshape(ntb, P, K * 3),
        })
    resB = run_bass_kernel_spmd(ncB, in_maps, list(range(N_CORES)))

    out = np.empty((M_TOTAL, 3), np.float32)
    for c in range(N_CORES):
        rdev = np.asarray(resB.results[c]["out"]).reshape(ntb * P * K, 3)
        out[orderB[c * M_CORE:(c + 1) * M_CORE]] = rdev[fposB[c]]
    return out

